# revision 60
# baseline (speedup 1.0000x reference)
"""AttentionBlock (GroupNorm + single-head 4096x4096 attention + residual) on 8 trn2 cores.

Sharding: core = 2*b + h. Data-parallel over batch (B=4), sequence-parallel over
query rows (2 halves of 2048). Each core receives its batch's x transposed to
[C, N] with token columns rotated so the core's own query tokens are columns
0..2047. K/V are computed for all 4096 tokens on both cores of a pair
(duplicated ~17% of FLOPs, no collectives).

Precision plan:
  - QKV + output projections run in bf16 (separate LDWEIGHTS + FWL, unlike
    f32r whose in-matmul 4-byte weight load can't overlap the previous MM).
  - The two big attention matmuls (S = K^T Q and O = V^T P) run in fp8e4 with
    perf_mode=DoubleRow: one MM contracts 256 (two 128-blocks), ~2x f32r.
  - exp is computed with a -3.0 shift so unnormalized P fits fp8e4 (TRN e4m3
    overflows to Inf at 256); the shift cancels exactly in O/r.
  - The V bias is NOT added to V: softmax weights sum to 1, so  sum_j p_j
    (v_j + bv)/r = O/r + bv, and bv rides into the output-projection bias
    (bp'' = bp + (bv + t@wv) @ wp). V is then a pure matmul output -> fp8.

Memory plan (per core, SBUF ~208KB/partition, all resident; no DRAM spill):
  x bf16 [C,4096] 32KB/p, x own-half f32 [C,2048] 32KB/p (residual),
  K^T fp8 pair-layout 16KB/p, Q^T fp8 8KB/p, V fp8 pair-layout 16KB/p,
  weights bf16 16KB/p.
GroupNorm is folded into the projections (w' = diag(s) w, bias' += t @ w).

PSUM: psBig 2x[128,1024] (4 banks: S pair-tiles / stage-B pair psums),
      psQuad 4x[128,512] (4 banks: psO accumulators, epilogue reuses them).
"""

import numpy as np
from contextlib import ExitStack

import concourse.bacc as bacc
import concourse.mybir as mybir
import concourse.tile as tile
from concourse.bass_utils import run_bass_kernel_spmd

F32 = mybir.dt.float32
F32R = mybir.dt.float32r
BF16 = mybir.dt.bfloat16
FP8 = mybir.dt.float8e4
AF = mybir.ActivationFunctionType
OP = mybir.AluOpType
DR = mybir.MatmulPerfMode.DoubleRow

B, HH, WW, C = 4, 64, 64, 512
NTOK = HH * WW          # 4096 tokens per batch
NOWN = NTOK // 2        # 2048 own query tokens per core
GROUPS = 32
CG = C // GROUPS        # 16 channels per group
EPS = 1e-5
CT = C // 128           # 4 channel tiles
QTOK = 1024             # token quarter
NQ = NTOK // QTOK       # 4 quarters
NPAIR = NTOK // 256     # 16 key-token pairs (256 tokens each)
IB = NOWN // 512        # 4 query i-blocks per core
SCALE = float(C) ** -0.5
ESHIFT = -3.0           # exp shift; cancels in O/r, keeps fp8 P < 240

_CACHE = {}


def _build_nc():
    if "nc" in _CACHE:
        return _CACHE["nc"]

    nc = bacc.Bacc(trn_type="TRN2")

    x8T = nc.dram_tensor("x8T", [C, NTOK], FP8, kind="ExternalInput")
    xoT = nc.dram_tensor("xoT", [C, NOWN], F32, kind="ExternalInput")
    w_ext = {
        n: nc.dram_tensor(n, [C, C], BF16, kind="ExternalInput")
        for n in ("wq", "wk", "wv", "wp")
    }
    b_ext = {
        n: nc.dram_tensor(n, [C], F32, kind="ExternalInput")
        for n in ("bq", "bk", "bv", "bp")
    }
    gamma_ext = nc.dram_tensor("gamma", [C], F32, kind="ExternalInput")
    beta_ext = nc.dram_tensor("beta", [C], F32, kind="ExternalInput")
    gsel_ext = nc.dram_tensor("gsel", [128, 8], F32, kind="ExternalInput")
    gselT_ext = nc.dram_tensor("gselT", [8, 128], F32, kind="ExternalInput")
    yT_ext = nc.dram_tensor("yT", [C, NOWN], F32, kind="ExternalOutput")

    with ExitStack() as ctx:
        tc = ctx.enter_context(tile.TileContext(nc))

        # ---- persistent pools ------------------------------------------------
        smalls = ctx.enter_context(tc.tile_pool(name="smalls", bufs=1))
        gnp = ctx.enter_context(tc.tile_pool(name="gnp", bufs=2))
        xbfp = ctx.enter_context(tc.tile_pool(name="xbfp", bufs=1))
        xop = ctx.enter_context(tc.tile_pool(name="xop", bufs=1))
        ktp = ctx.enter_context(tc.tile_pool(name="ktp", bufs=1))
        qp = ctx.enter_context(tc.tile_pool(name="qp", bufs=1))
        vp = ctx.enter_context(tc.tile_pool(name="vp", bufs=1))
        wpp = ctx.enter_context(tc.tile_pool(name="wpp", bufs=1))

        psBig = ctx.enter_context(tc.tile_pool(name="psBig", bufs=2, space="PSUM"))
        psQuad = ctx.enter_context(tc.tile_pool(name="psQuad", bufs=4, space="PSUM"))

        # ---- small constants -------------------------------------------------
        ones1_f = smalls.tile([1, 128], F32, tag="ones1_f")
        nc.vector.memset(ones1_f, 1.0)
        ones_f = smalls.tile([128, 1], F32, tag="ones_f")
        nc.vector.memset(ones_f, 1.0)
        ones_r = smalls.tile([128, 1], F32R, tag="ones_r")
        nc.vector.tensor_copy(ones_r[:], ones_f[:])
        eps_row = smalls.tile([8, 1], F32, tag="eps_row")
        nc.vector.memset(eps_row, EPS)
        zbias = smalls.tile([128, 1], F32, tag="zbias")
        nc.vector.memset(zbias, 0.0)
        ebias = smalls.tile([128, 1], F32, tag="ebias")
        nc.vector.memset(ebias, ESHIFT)

        ones8 = smalls.tile([128, 1], FP8, tag="ones8")
        nc.vector.memset(ones8, 1.0)

        gsel_sb = smalls.tile([128, 8], F32, tag="gsel")
        nc.sync.dma_start(gsel_sb[:], gsel_ext[:])
        gselT_sb = smalls.tile([8, 128], F32, tag="gselT")
        nc.sync.dma_start(gselT_sb[:], gselT_ext[:])

        def col_tiles(ext, tag):
            # one strided DMA for all CT column tiles: [p, t] <- flat t*128+p
            v = ext.rearrange("(t p) -> p t", p=128)
            s = smalls.tile([128, CT], F32, tag=tag)
            nc.sync.dma_start(s[:], v)
            return [s[:, t : t + 1] for t in range(CT)]

        gamma_t = col_tiles(gamma_ext, "gamma")
        beta_t = col_tiles(beta_ext, "beta")
        bq_t = col_tiles(b_ext["bq"], "bq")
        bk_t = col_tiles(b_ext["bk"], "bk")
        bv_t = col_tiles(b_ext["bv"], "bv")
        bp_t = col_tiles(b_ext["bp"], "bp")

        # ---- resident tensors ------------------------------------------------
        # x fp8: x8t[q] flat [p, ci*1024 + t]; channel = ci*128 + p
        # (ci = 2*ci2 + i gives the DoubleRow pair layout per ci2 for free)
        x8t = {
            q: xbfp.tile([128, 4 * QTOK], FP8, tag=f"x8{q}", name=f"x8{q}")
            for q in range(NQ)
        }
        # raw f32 own-half x (residual source): [p, co*2048 + tok]
        xo_all = xop.tile([128, CT * NOWN], F32, tag="xo", name="xo")
        # K^T fp8 pair-layout: KT8[ci2] flat [p, half*4096 + j];
        # channel = ci2*256 + half*128 + p
        KT8 = [
            ktp.tile([128, 2 * NTOK], FP8, tag=f"kt{c}", name=f"kt{c}")
            for c in range(2)
        ]
        # Q^T fp8 pair-layout: Q8[ci2] flat [p, half*2048 + n] over own queries
        Q8 = [
            qp.tile([128, 2 * NOWN], FP8, tag=f"q8{c}", name=f"q8{c}")
            for c in range(2)
        ]
        # V fp8 pair-layout: V8[pair] flat [p, i*512 + c]; token = pair*256+i*128+p
        V8 = [
            vp.tile([128, 1024], FP8, tag=f"v8{j}", name=f"v8{j}")
            for j in range(NPAIR)
        ]
        # bf16 weights: w_r[name][ci] = [128, C]
        w_r = {}

        # ---- stage A: DMA + groupnorm statistics -----------------------------
        with nc.named_scope("stats"):
            stats_t = [
                gnp.tile([128, 2 * NQ, 6], F32, tag=f"stats{t}", name=f"stats{t}")
                for t in range(CT)
            ]
            # one 3D DMA per quarter: [p, ci, t] <- x8T[ci*128+p, ...]
            # Stats are split: the scalar engine (idle here) takes the 5
            # earliest-arriving chunks via activation accum_out (sum of x and
            # x^2); the DVE bn_stats the rest. Cuts ~10us off the serial
            # stats tail that gates all projections.
            SC_CHUNKS = {(0, 0), (0, 1), (0, 2), (0, 3), (1, 3)}
            ssum, ssq = {}, {}
            junkp = ctx.enter_context(tc.tile_pool(name="junk", bufs=2))
            x8src = x8T.rearrange("(a p) t -> p a t", p=128)
            for q in range(NQ):
                nc.sync.dma_start(
                    x8t[q][:].rearrange("p (a t) -> p a t", a=CT),
                    x8src[:, :, q * QTOK : (q + 1) * QTOK],
                )
                # HAM warm-up: dummy row-sum matmuls paced by the DMA
                # arrivals keep the PE's activity monitor at K=8/8 through
                # stage A, so stage B doesn't start at the 1.2 GHz cold clock.
                for k in range(8):
                    wps = psQuad.tile([1, 512], F32, tag="psQ", name=f"warm{q}{k}")
                    nc.tensor.matmul(
                        wps[:],
                        ones8[:],
                        x8t[q][:, k * 512 : (k + 1) * 512],
                        start=True,
                        stop=True,
                    )
                for t in range(CT):
                    sl = x8t[q][:, t * QTOK : (t + 1) * QTOK]
                    if (q, t) in SC_CHUNKS:
                        s1 = gnp.tile([128, 1], F32, tag=f"ss{q}{t}", name=f"ss{q}{t}")
                        s2 = gnp.tile([128, 1], F32, tag=f"sq{q}{t}", name=f"sq{q}{t}")
                        j1 = junkp.tile([128, QTOK], F32, tag="junk")
                        nc.scalar.activation(
                            j1[:], sl, AF.Identity, bias=zbias[:], accum_out=s1[:]
                        )
                        j2 = junkp.tile([128, QTOK], F32, tag="junk")
                        nc.scalar.activation(
                            j2[:], sl, AF.Square, bias=zbias[:], accum_out=s2[:]
                        )
                        ssum[q, t] = s1
                        ssq[q, t] = s2
                    else:
                        nc.vector.bn_stats(stats_t[t][:, 2 * q, :], sl[:, 0:512])
                        nc.vector.bn_stats(
                            stats_t[t][:, 2 * q + 1, :], sl[:, 512:1024]
                        )

            # weights land during the stats compute: one 3D DMA per tensor.
            # Issued BEFORE the residual xo (needed only at the epilogues) so
            # the rank1/scale/w8 chain isn't stuck behind a 4MB transfer.
            for n in ("wk", "wq", "wv", "wp"):
                wall = wpp.tile([128, CT * C], BF16, tag=f"w{n}")
                nc.sync.dma_start(
                    wall[:].rearrange("p (a c) -> p a c", a=CT),
                    w_ext[n].rearrange("(a p) c -> p a c", p=128),
                )
                w_r[n] = [wall[:, ci * C : (ci + 1) * C] for ci in range(CT)]
            wp_r = w_r["wp"]

            # residual x: one 3D DMA  [p, co, tok] <- xoT[co*128+p, tok]
            nc.sync.dma_start(
                xo_all[:].rearrange("p (a t) -> p a t", a=CT),
                xoT.rearrange("(a p) t -> p a t", p=128),
            )

            packed = gnp.tile([128, 2 * CT], F32, tag="packed")
            for t in range(CT):
                # merge DVE bn_stats (N_d tokens) with scalar accum sums
                nsc = sum(1 for q in range(NQ) if (q, t) in SC_CHUNKS)
                n_d = NTOK - nsc * QTOK
                # aggregate only the DVE-written slots (bn_aggr's variance
                # merge breaks on zero-count slots); scalar chunks are a
                # prefix of the quarters, so valid slots are contiguous.
                mv = gnp.tile([128, 2], F32, tag="mv")
                nc.vector.bn_aggr(mv[:], stats_t[t][:, 2 * nsc : 2 * NQ, :])
                tmp = gnp.tile([128, 1], F32, tag="tmp")
                nc.vector.tensor_mul(tmp[:], mv[:, 0:1], mv[:, 0:1])
                e2d = gnp.tile([128, 1], F32, tag="e2d")
                nc.vector.tensor_add(e2d[:], mv[:, 1:2], tmp[:])
                qs = [q for q in range(NQ) if (q, t) in SC_CHUNKS]
                s1, s2 = ssum[qs[0], t], ssq[qs[0], t]
                for q in qs[1:]:
                    s1b = gnp.tile([128, 1], F32, tag="s1b")
                    nc.vector.tensor_add(s1b[:], s1[:], ssum[q, t][:])
                    s2b = gnp.tile([128, 1], F32, tag="s2b")
                    nc.vector.tensor_add(s2b[:], s2[:], ssq[q, t][:])
                    s1, s2 = s1b, s2b
                ma = gnp.tile([128, 1], F32, tag="ma")
                nc.vector.tensor_scalar_mul(ma[:], mv[:, 0:1], float(n_d) / NTOK)
                mb = gnp.tile([128, 1], F32, tag="mb")
                nc.vector.tensor_scalar_mul(mb[:], s1[:], 1.0 / NTOK)
                nc.vector.tensor_add(packed[:, 2 * t : 2 * t + 1], ma[:], mb[:])
                ea = gnp.tile([128, 1], F32, tag="ea")
                nc.vector.tensor_scalar_mul(ea[:], e2d[:], float(n_d) / NTOK)
                eb = gnp.tile([128, 1], F32, tag="eb")
                nc.vector.tensor_scalar_mul(eb[:], s2[:], 1.0 / NTOK)
                nc.vector.tensor_add(
                    packed[:, 2 * t + 1 : 2 * t + 2], ea[:], eb[:]
                )

            g_ps = psQuad.tile([8, 2 * CT], F32, tag="psQ", name="g_ps")
            nc.tensor.matmul(g_ps[:], gsel_sb[:], packed[:], start=True, stop=True)
            stat2 = gnp.tile([8, 2 * CT], F32, tag="stat2")
            nc.vector.tensor_scalar_mul(stat2[:], g_ps[:], 1.0 / CG)
            s2v = stat2.rearrange("g (t two) -> g t two", two=2)
            mu_v = s2v[:, :, 0]
            e2_v = s2v[:, :, 1]
            musq = gnp.tile([8, CT], F32, tag="musq")
            nc.vector.tensor_mul(musq[:], mu_v, mu_v)
            var = gnp.tile([8, CT], F32, tag="var")
            nc.vector.tensor_sub(var[:], e2_v, musq[:])
            # rstd = exp(-0.5*ln(var+eps)): stays on the natural_log_exp
            # activation table set, the same one attention's exp uses — the
            # kernel then needs only ONE ~2.7us ACT_TABLE_LOAD total.
            lnv = gnp.tile([8, CT], F32, tag="lnv")
            nc.scalar.activation(lnv[:], var[:], AF.Ln, bias=eps_row[:], scale=1.0)
            rstd = gnp.tile([8, CT], F32, tag="rstd")
            nc.scalar.activation(rstd[:], lnv[:], AF.Exp, bias=0.0, scale=-0.5)

            scale_t, shift_t, shift_bf = [], [], []
            for t in range(CT):
                cat2 = gnp.tile([8, 2], F32, tag="cat2")
                nc.vector.tensor_copy(cat2[:, 0:1], mu_v[:, t : t + 1])
                nc.vector.tensor_copy(cat2[:, 1:2], rstd[:, t : t + 1])
                bc_ps = psQuad.tile([128, 2], F32, tag="psQ", name=f"bc{t}")
                nc.tensor.matmul(bc_ps[:], gselT_sb[:], cat2[:], start=True, stop=True)
                sc = gnp.tile([128, 1], F32, tag=f"scale{t}")
                nc.vector.tensor_mul(sc[:], bc_ps[:, 1:2], gamma_t[t])
                tmp2 = gnp.tile([128, 1], F32, tag="tmp2")
                nc.vector.tensor_mul(tmp2[:], bc_ps[:, 0:1], sc[:])
                sh = gnp.tile([128, 1], F32, tag=f"shift{t}")
                nc.vector.tensor_sub(sh[:], beta_t[t], tmp2[:])
                shb = gnp.tile([128, 1], BF16, tag=f"shiftb{t}")
                nc.vector.tensor_copy(shb[:], sh[:])
                scale_t.append(sc)
                shift_t.append(sh)
                shift_bf.append(shb)

            # ---- fold groupnorm into the projections ------------------------
            # xn = s*x + t  =>  xn @ w = x @ (diag(s) w) + (t @ w).

            def rank1_bias(wname, b_tiles, shvec, tag):
                """per-co bias tiles: b[co] + sum_ci shvec[ci] @ w[ci, co]"""
                out = []
                for co in range(CT):
                    ps = psQuad.tile([128, 1], F32, tag="psQ", name=f"r1{tag}{co}")
                    for ci in range(CT):
                        nc.tensor.matmul(
                            ps[:],
                            w_r[wname][ci][:, co * 128 : (co + 1) * 128],
                            shvec[ci][:],
                            start=(ci == 0),
                            stop=(ci == CT - 1),
                        )
                    bt = smalls.tile([128, 1], F32, tag=f"bfold{tag}{co}")
                    nc.vector.tensor_add(bt[:], b_tiles[co], ps[:])
                    out.append(bt)
                return out

            biask_t = rank1_bias("wk", bk_t, shift_bf, "k")
            biasq_t = rank1_bias("wq", bq_t, shift_bf, "q")
            # V bias rides through the softmax (weights sum to 1):
            # bp'' = bp + (bv + t @ wv) @ wp
            bvp_t = rank1_bias("wv", bv_t, shift_bf, "v")
            bvp_bf = []
            for ci in range(CT):
                bb = gnp.tile([128, 1], BF16, tag=f"bvpb{ci}")
                nc.vector.tensor_copy(bb[:], bvp_t[ci][:])
                bvp_bf.append(bb)
            bpp_t = rank1_bias("wp", bp_t, bvp_bf, "p")
            # fold the (bp + bv'@wp) bias into the residual once, so the
            # per-block epilogue needs only y = y1 + xr' (plain add, no STT).
            # On the DVE: the scalar FIFO feeds stage B's K/Q activations next.
            for co in range(CT):
                sl = xo_all[:, co * NOWN : (co + 1) * NOWN]
                nc.vector.tensor_scalar_add(sl, sl, bpp_t[co][:])

            # in-place row-scale of wq/wk/wv (after all rank-1 reads), then
            # fp8 DoubleRow conversion: w8[n][ci2] flat [p, i*512 + co];
            # input channel = ci2*256 + i*128 + p. wk goes first so stage B's
            # K matmuls can start while wq/wv are still being converted.
            w8 = {}

            def scale_w(n):
                for ci in range(CT):
                    nc.vector.tensor_scalar_mul(
                        w_r[n][ci][:], w_r[n][ci][:], scale_t[ci][:]
                    )

            def conv_w8(n):
                w8[n] = []
                for c in range(2):
                    t8 = wpp.tile([128, 1024], FP8, tag=f"w8{n}{c}")
                    for i in range(2):
                        nc.scalar.activation(
                            t8[:, i * 512 : (i + 1) * 512],
                            w_r[n][2 * c + i][:],
                            AF.Identity,
                            bias=zbias[:],
                            scale=1.0,
                        )
                    w8[n].append(t8)

            scale_w("wk")
            conv_w8("wk")
            scale_w("wq")
            conv_w8("wq")
            scale_w("wv")

        # ---- stage B: QKV projections (K/Q fp8 DoubleRow, V bf16) ------------
        with nc.named_scope("qkv"):
            w8v = {
                n: [
                    w8[n][c][:].rearrange("p (two co) -> p two co", two=2)
                    for c in range(2)
                ]
                for n in ("wk", "wq")
            }
            x8v = {
                (q, c): x8t[q][:, c * 2 * QTOK : (c + 1) * 2 * QTOK].rearrange(
                    "p (two t) -> p two t", two=2
                )
                for q in range(NQ)
                for c in range(2)
            }
            for q in range(NQ):
                # K^T: per co, both 512-chunks into one [128,1024] psum
                for co in range(CT):
                    psB = psBig.tile([128, 1024], F32, tag="psB")
                    for nch in range(2):
                        for c in range(2):
                            nc.tensor.matmul(
                                psB[:, nch * 512 : (nch + 1) * 512],
                                w8v["wk"][c][:, :, co * 128 : (co + 1) * 128],
                                x8v[q, c][:, :, nch * 512 : (nch + 1) * 512],
                                start=(c == 0),
                                stop=(c == 1),
                                perf_mode=DR,
                            )
                    ci2, half = co // 2, co % 2
                    dst = KT8[ci2][
                        :, half * NTOK + q * QTOK : half * NTOK + (q + 1) * QTOK
                    ]
                    nc.scalar.activation(
                        dst, psB[:], AF.Identity, bias=biask_t[co][:], scale=1.0
                    )
                # Q^T: own-half quarters only
                if q < 2:
                    for co in range(CT):
                        psB = psBig.tile([128, 1024], F32, tag="psB")
                        for nch in range(2):
                            for c in range(2):
                                nc.tensor.matmul(
                                    psB[:, nch * 512 : (nch + 1) * 512],
                                    w8v["wq"][c][:, :, co * 128 : (co + 1) * 128],
                                    x8v[q, c][:, :, nch * 512 : (nch + 1) * 512],
                                    start=(c == 0),
                                    stop=(c == 1),
                                    perf_mode=DR,
                                )
                        ci2, half = co // 2, co % 2
                        dst = Q8[ci2][
                            :, half * NOWN + q * QTOK : half * NOWN + (q + 1) * QTOK
                        ]
                        nc.scalar.activation(
                            dst, psB[:], AF.Identity, bias=biasq_t[co][:], scale=1.0
                        )
                # V: two 128-token tiles into one [128,1024] psum -> V8[pair]
                # lhsT is fp8 x with bf16 wv rhs (mixed non-fp32 dtypes)
                for jt2 in range(4):
                    psB = psBig.tile([128, 1024], F32, tag="psB")
                    for half2 in range(2):
                        jt = jt2 * 2 + half2
                        for ci in range(CT):
                            nc.tensor.matmul(
                                psB[:, half2 * 512 : (half2 + 1) * 512],
                                x8t[q][
                                    :,
                                    ci * QTOK + jt * 128 : ci * QTOK + (jt + 1) * 128,
                                ],
                                w_r["wv"][ci][:],
                                start=(ci == 0),
                                stop=(ci == CT - 1),
                            )
                    pair = q * 4 + jt2
                    # on the DVE: keeps the scalar FIFO free so the first
                    # attention exps aren't starved behind stage-B activations
                    nc.vector.tensor_copy(V8[pair][:], psB[:])

        # ---- stage C: attention + projection ---------------------------------
        with (
            tc.tile_pool(name="pt", bufs=8) as ptp,
            tc.tile_pool(name="osb", bufs=4) as osbp,
            tc.tile_pool(name="ysb", bufs=3) as ysbp,
            tc.tile_pool(name="yraw", bufs=4) as yrawp,
            tc.tile_pool(name="racc", bufs=2) as raccp,
            tc.tile_pool(name="rsb", bufs=2) as rsbp,
            nc.named_scope("attn"),
        ):
            kt3 = [
                KT8[c][:].rearrange("p (two j) -> p two j", two=2) for c in range(2)
            ]
            v3 = [
                V8[j][:].rearrange("p (two c) -> p two c", two=2)
                for j in range(NPAIR)
            ]

            def emit_o(state, pair, pt_t):
                # psO tiles are allocated lazily at the first emit_o so the
                # previous block's epilogue PSUM allocations (emitted at
                # pair==1) precede them in pool order — otherwise the pool's
                # FIFO buffer reuse creates an allocation-order deadlock.
                if state["psO_t"] is None:
                    ib = state["ib"]
                    state["psO_t"] = [
                        psQuad.tile([128, 512], F32, tag="psQ", name=f"psO_{ib}_{i}")
                        for i in range(CT)
                    ]
                psO_t = state["psO_t"]
                pt3 = pt_t[:].rearrange("p (two n) -> p two n", two=2)
                for ct in range(CT):
                    nc.tensor.matmul(
                        psO_t[ct][:],
                        v3[pair][:, :, ct * 128 : (ct + 1) * 128],
                        pt3,
                        start=(pair == 0),
                        stop=(pair == NPAIR - 1),
                        perf_mode=DR,
                    )

            def emit_epilogue_a(state):
                """r chain + O drain + projection for a finished block.

                Engine placement is deliberate: osb/yraw go on the DVE (they
                are data-ready when emitted; on the scalar FIFO they would
                delay the next block's exp), rinv uses the fast approx so it
                finishes before the PE reaches the rb broadcast matmul."""
                ib = state["ib"]
                racc, psO_t = state["racc"], state["psO_t"]

                psr = psQuad.tile([1, 512], F32, tag="psQ", name=f"psr{ib}")
                nc.tensor.matmul(psr[:], ones_r[:], racc[:], start=True, stop=False)
                nc.tensor.matmul(
                    psr[:], ones_r[:], state["racc2"][:], start=False, stop=True
                )

                osb = []
                for ct in range(CT):
                    o_t = osbp.tile([128, 512], BF16, tag="osb")
                    nc.scalar.activation(
                        o_t[:], psO_t[ct][:], AF.Identity, bias=zbias[:], scale=1.0
                    )
                    osb.append(o_t)

                rinv = rsbp.tile([1, 512], F32, tag="rinv")
                rscratch = rsbp.tile([1, 512], F32, tag="rscr")
                nc.vector.reciprocal_approx_accurate(
                    rinv[:], psr[:], rscratch[:]
                )

                yraw = []
                for co in range(CT):
                    psY = psQuad.tile([128, 512], F32, tag="psQ", name=f"psY{ib}{co}")
                    for ci in range(CT):
                        nc.tensor.matmul(
                            psY[:],
                            wp_r[ci][:, co * 128 : (co + 1) * 128],
                            osb[ci][:],
                            start=(ci == 0),
                            stop=(ci == CT - 1),
                        )
                    yr = yrawp.tile([128, 512], F32, tag="yraw")
                    nc.scalar.activation(
                        yr[:], psY[:], AF.Identity, bias=zbias[:], scale=1.0
                    )
                    yraw.append(yr)
                state["yraw"] = yraw

                rb_ps = psQuad.tile([128, 512], F32, tag="psQ", name=f"rb{ib}")
                nc.tensor.matmul(rb_ps[:], ones1_f[:], rinv[:], start=True, stop=True)
                rb_sb = rsbp.tile([128, 512], F32, tag="rb_sb")
                nc.vector.tensor_copy(rb_sb[:], rb_ps[:])
                state["rb_sb"] = rb_sb

            def emit_epilogue_b(state):
                """normalize + bias + residual + output DMA (rb surely ready)."""
                ib = state["ib"]
                i0 = ib * 512
                rb_sb, yraw = state["rb_sb"], state["yraw"]
                for co in range(CT):
                    xr = xo_all[:, co * NOWN + i0 : co * NOWN + i0 + 512]
                    y1_t = ysbp.tile([128, 512], F32, tag="y1sb")
                    nc.vector.tensor_mul(y1_t[:], yraw[co][:], rb_sb[:])
                    y_t = ysbp.tile([128, 512], F32, tag="ysb")
                    nc.vector.tensor_add(y_t[:], y1_t[:], xr)
                    nc.sync.dma_start(
                        yT_ext[co * 128 : (co + 1) * 128, i0 : i0 + 512], y_t[:]
                    )

            # 2-deep software pipeline ACROSS block boundaries: the last two
            # O groups of block b interleave with block b+1's first S groups,
            # so the PE never runs an S-only (exp-gated) stretch.
            done_state = None
            pending = []  # [(state, pair, pt_t)]
            for ib in range(IB):
                qrhs = [
                    Q8[c][:].rearrange("p (two n) -> p two n", two=2)[
                        :, :, ib * 512 : (ib + 1) * 512
                    ]
                    for c in range(2)
                ]
                state = {
                    "ib": ib,
                    "psO_t": None,
                    "racc": raccp.tile([128, 512], F32R, tag="racc", name=f"racc{ib}"),
                    "racc2": raccp.tile(
                        [128, 512], F32R, tag="racc2", name=f"racc2_{ib}"
                    ),
                }
                racc = state["racc"]

                for pair in range(NPAIR):
                    psS2 = psBig.tile([128, 1024], F32, tag="psB")
                    for half in range(2):
                        j = pair * 2 + half
                        for ci2 in range(2):
                            nc.tensor.matmul(
                                psS2[:, half * 512 : (half + 1) * 512],
                                kt3[ci2][:, :, j * 128 : (j + 1) * 128],
                                qrhs[ci2],
                                start=(ci2 == 0),
                                stop=(ci2 == 1),
                                perf_mode=DR,
                            )
                    # epilogue BEFORE the pop: its PSUM allocations (psr/psY/rb)
                    # must precede the next block's psO allocations in pool
                    # order, or the pool's FIFO buffer reuse stalls the PE a
                    # whole block waiting on them.
                    if pair == 2 and done_state is not None:
                        # previous block's epilogue (its last O landed at pair 1)
                        emit_epilogue_a(done_state)
                    elif pair == 7 and done_state is not None:
                        emit_epilogue_b(done_state)
                        done_state = None
                    if len(pending) >= 2:
                        emit_o(*pending.pop(0))
                    pt_t = ptp.tile([128, 1024], FP8, tag="pt")
                    nc.scalar.activation(
                        pt_t[:], psS2[:], AF.Exp, bias=ebias[:], scale=SCALE
                    )
                    # two running sums, one per engine, merged in the epilogue:
                    # DVE accumulates half 0, GpSimd half 1 (fp8 reads)
                    racc2 = state["racc2"]
                    if pair == 0:
                        nc.vector.tensor_copy(racc[:], pt_t[:, 0:512])
                        nc.gpsimd.tensor_copy(racc2[:], pt_t[:, 512:1024])
                    else:
                        nc.vector.tensor_add(racc[:], racc[:], pt_t[:, 0:512])
                        nc.gpsimd.tensor_add(racc2[:], racc2[:], pt_t[:, 512:1024])
                    pending.append((state, pair, pt_t))
                done_state = state
            for item in pending:
                emit_o(*item)
            emit_epilogue_a(done_state)
            emit_epilogue_b(done_state)

    nc.compile()
    _CACHE["nc"] = nc
    return nc


def make_in_maps(x, gamma, beta, wq, bq, wk, bk, wv, bv, wp, bp):
    import ml_dtypes

    bf16 = ml_dtypes.bfloat16
    x = np.asarray(x, dtype=np.float32)
    gsel = np.zeros((128, 8), np.float32)
    for p in range(128):
        gsel[p, p // CG % 8] = 1.0
    gselT = np.ascontiguousarray(gsel.T)

    shared = {
        "wq": np.asarray(wq, np.float32).astype(bf16),
        "wk": np.asarray(wk, np.float32).astype(bf16),
        "wv": np.asarray(wv, np.float32).astype(bf16),
        "wp": np.asarray(wp, np.float32).astype(bf16),
        "bq": np.asarray(bq, np.float32),
        "bk": np.asarray(bk, np.float32),
        "bv": np.asarray(bv, np.float32),
        "bp": np.asarray(bp, np.float32),
        "gamma": np.asarray(gamma, np.float32),
        "beta": np.asarray(beta, np.float32),
        "gsel": gsel,
        "gselT": gselT,
    }

    in_maps = []
    for core in range(8):
        b, h = core // 2, core % 2
        xT_b = np.ascontiguousarray(x[b].reshape(NTOK, C).T)  # [C, NTOK]
        if h == 1:
            xT_b = np.ascontiguousarray(
                np.concatenate([xT_b[:, NOWN:], xT_b[:, :NOWN]], axis=1)
            )
        in_maps.append(
            {
                # |x| < 240, so OCP e4m3fn bytes == TRN fp8e4 bytes
                "x8T": xT_b.astype(ml_dtypes.float8_e4m3fn),
                "xoT": np.ascontiguousarray(xT_b[:, :NOWN]),
                **shared,
            }
        )
    return in_maps


def kernel(x, gamma, beta, wq, bq, wk, bk, wv, bv, wp, bp):
    nc = _build_nc()
    in_maps = make_in_maps(x, gamma, beta, wq, bq, wk, bk, wv, bv, wp, bp)
    _CACHE["in_maps"] = in_maps

    res = run_bass_kernel_spmd(nc, in_maps, core_ids=list(range(8)))

    y = np.empty((B, NTOK, C), np.float32)
    for core in range(8):
        b, h = core // 2, core % 2
        yT = res.results[core]["yT"]  # [C, NOWN]
        y[b, h * NOWN : (h + 1) * NOWN, :] = yT.T
    return y.reshape(B, HH, WW, C)


# revision 61
# speedup vs baseline: 1.0119x; 1.0119x over previous
"""AttentionBlock (GroupNorm + single-head 4096x4096 attention + residual) on 8 trn2 cores.

Sharding: core = 2*b + h. Data-parallel over batch (B=4), sequence-parallel over
query rows (2 halves of 2048). Each core receives its batch's x transposed to
[C, N] with token columns rotated so the core's own query tokens are columns
0..2047. K/V are computed for all 4096 tokens on both cores of a pair
(duplicated ~17% of FLOPs, no collectives).

Precision plan:
  - QKV + output projections run in bf16 (separate LDWEIGHTS + FWL, unlike
    f32r whose in-matmul 4-byte weight load can't overlap the previous MM).
  - The two big attention matmuls (S = K^T Q and O = V^T P) run in fp8e4 with
    perf_mode=DoubleRow: one MM contracts 256 (two 128-blocks), ~2x f32r.
  - exp is computed with a -3.0 shift so unnormalized P fits fp8e4 (TRN e4m3
    overflows to Inf at 256); the shift cancels exactly in O/r.
  - The V bias is NOT added to V: softmax weights sum to 1, so  sum_j p_j
    (v_j + bv)/r = O/r + bv, and bv rides into the output-projection bias
    (bp'' = bp + (bv + t@wv) @ wp). V is then a pure matmul output -> fp8.

Memory plan (per core, SBUF ~208KB/partition, all resident; no DRAM spill):
  x bf16 [C,4096] 32KB/p, x own-half f32 [C,2048] 32KB/p (residual),
  K^T fp8 pair-layout 16KB/p, Q^T fp8 8KB/p, V fp8 pair-layout 16KB/p,
  weights bf16 16KB/p.
GroupNorm is folded into the projections (w' = diag(s) w, bias' += t @ w).

PSUM: psBig 2x[128,1024] (4 banks: S pair-tiles / stage-B pair psums),
      psQuad 4x[128,512] (4 banks: psO accumulators, epilogue reuses them).
"""

import numpy as np
from contextlib import ExitStack

import concourse.bacc as bacc
import concourse.mybir as mybir
import concourse.tile as tile
from concourse.bass_utils import run_bass_kernel_spmd

F32 = mybir.dt.float32
F32R = mybir.dt.float32r
BF16 = mybir.dt.bfloat16
FP8 = mybir.dt.float8e4
AF = mybir.ActivationFunctionType
OP = mybir.AluOpType
DR = mybir.MatmulPerfMode.DoubleRow

B, HH, WW, C = 4, 64, 64, 512
NTOK = HH * WW          # 4096 tokens per batch
NOWN = NTOK // 2        # 2048 own query tokens per core
GROUPS = 32
CG = C // GROUPS        # 16 channels per group
EPS = 1e-5
CT = C // 128           # 4 channel tiles
QTOK = 1024             # token quarter
NQ = NTOK // QTOK       # 4 quarters
NPAIR = NTOK // 256     # 16 key-token pairs (256 tokens each)
IB = NOWN // 512        # 4 query i-blocks per core
SCALE = float(C) ** -0.5
ESHIFT = -3.0           # exp shift; cancels in O/r, keeps fp8 P < 240

_CACHE = {}


def _build_nc():
    if "nc" in _CACHE:
        return _CACHE["nc"]

    nc = bacc.Bacc(trn_type="TRN2")

    x8T = nc.dram_tensor("x8T", [C, NTOK], FP8, kind="ExternalInput")
    xoT = nc.dram_tensor("xoT", [C, NOWN], F32, kind="ExternalInput")
    w_ext = {
        n: nc.dram_tensor(n, [C, C], BF16, kind="ExternalInput")
        for n in ("wq", "wk", "wv", "wp")
    }
    b_ext = {
        n: nc.dram_tensor(n, [C], F32, kind="ExternalInput")
        for n in ("bq", "bk", "bv", "bp")
    }
    gamma_ext = nc.dram_tensor("gamma", [C], F32, kind="ExternalInput")
    beta_ext = nc.dram_tensor("beta", [C], F32, kind="ExternalInput")
    gsel_ext = nc.dram_tensor("gsel", [128, 8], F32, kind="ExternalInput")
    gselT_ext = nc.dram_tensor("gselT", [8, 128], F32, kind="ExternalInput")
    yT_ext = nc.dram_tensor("yT", [C, NOWN], F32, kind="ExternalOutput")

    with ExitStack() as ctx:
        tc = ctx.enter_context(tile.TileContext(nc))

        # ---- persistent pools ------------------------------------------------
        smalls = ctx.enter_context(tc.tile_pool(name="smalls", bufs=1))
        gnp = ctx.enter_context(tc.tile_pool(name="gnp", bufs=2))
        xbfp = ctx.enter_context(tc.tile_pool(name="xbfp", bufs=1))
        xop = ctx.enter_context(tc.tile_pool(name="xop", bufs=1))
        ktp = ctx.enter_context(tc.tile_pool(name="ktp", bufs=1))
        qp = ctx.enter_context(tc.tile_pool(name="qp", bufs=1))
        vp = ctx.enter_context(tc.tile_pool(name="vp", bufs=1))
        wpp = ctx.enter_context(tc.tile_pool(name="wpp", bufs=1))

        psBig = ctx.enter_context(tc.tile_pool(name="psBig", bufs=2, space="PSUM"))
        psQuad = ctx.enter_context(tc.tile_pool(name="psQuad", bufs=4, space="PSUM"))

        # ---- small constants -------------------------------------------------
        ones1_f = smalls.tile([1, 128], F32, tag="ones1_f")
        nc.vector.memset(ones1_f, 1.0)
        ones_f = smalls.tile([128, 1], F32, tag="ones_f")
        nc.vector.memset(ones_f, 1.0)
        ones_r = smalls.tile([128, 1], F32R, tag="ones_r")
        nc.vector.tensor_copy(ones_r[:], ones_f[:])
        eps_row = smalls.tile([8, 1], F32, tag="eps_row")
        nc.vector.memset(eps_row, EPS)
        zbias = smalls.tile([128, 1], F32, tag="zbias")
        nc.vector.memset(zbias, 0.0)
        ebias = smalls.tile([128, 1], F32, tag="ebias")
        nc.vector.memset(ebias, ESHIFT)

        ones8 = smalls.tile([128, 1], FP8, tag="ones8")
        nc.vector.memset(ones8, 1.0)

        gsel_sb = smalls.tile([128, 8], F32, tag="gsel")
        nc.sync.dma_start(gsel_sb[:], gsel_ext[:])
        gselT_sb = smalls.tile([8, 128], F32, tag="gselT")
        nc.sync.dma_start(gselT_sb[:], gselT_ext[:])

        def col_tiles(ext, tag):
            # one strided DMA for all CT column tiles: [p, t] <- flat t*128+p
            v = ext.rearrange("(t p) -> p t", p=128)
            s = smalls.tile([128, CT], F32, tag=tag)
            nc.sync.dma_start(s[:], v)
            return [s[:, t : t + 1] for t in range(CT)]

        gamma_t = col_tiles(gamma_ext, "gamma")
        beta_t = col_tiles(beta_ext, "beta")
        bq_t = col_tiles(b_ext["bq"], "bq")
        bk_t = col_tiles(b_ext["bk"], "bk")
        bv_t = col_tiles(b_ext["bv"], "bv")
        bp_t = col_tiles(b_ext["bp"], "bp")

        # ---- resident tensors ------------------------------------------------
        # x fp8: x8t[q] flat [p, ci*1024 + t]; channel = ci*128 + p
        # (ci = 2*ci2 + i gives the DoubleRow pair layout per ci2 for free)
        x8t = {
            q: xbfp.tile([128, 4 * QTOK], FP8, tag=f"x8{q}", name=f"x8{q}")
            for q in range(NQ)
        }
        # raw f32 own-half x (residual source): [p, co*2048 + tok]
        xo_all = xop.tile([128, CT * NOWN], F32, tag="xo", name="xo")
        # K^T fp8 pair-layout: KT8[ci2] flat [p, half*4096 + j];
        # channel = ci2*256 + half*128 + p
        KT8 = [
            ktp.tile([128, 2 * NTOK], FP8, tag=f"kt{c}", name=f"kt{c}")
            for c in range(2)
        ]
        # Q^T fp8 pair-layout: Q8[ci2] flat [p, half*2048 + n] over own queries
        Q8 = [
            qp.tile([128, 2 * NOWN], FP8, tag=f"q8{c}", name=f"q8{c}")
            for c in range(2)
        ]
        # V fp8 pair-layout: V8[pair] flat [p, i*512 + c]; token = pair*256+i*128+p
        V8 = [
            vp.tile([128, 1024], FP8, tag=f"v8{j}", name=f"v8{j}")
            for j in range(NPAIR)
        ]
        # bf16 weights: w_r[name][ci] = [128, C]
        w_r = {}

        # ---- stage A: DMA + groupnorm statistics -----------------------------
        with nc.named_scope("stats"):
            stats_t = [
                gnp.tile([128, 2 * NQ, 6], F32, tag=f"stats{t}", name=f"stats{t}")
                for t in range(CT)
            ]
            # one 3D DMA per quarter: [p, ci, t] <- x8T[ci*128+p, ...]
            # Stats are split: the scalar engine (idle here) takes the 5
            # earliest-arriving chunks via activation accum_out (sum of x and
            # x^2); the DVE bn_stats the rest. Cuts ~10us off the serial
            # stats tail that gates all projections.
            SC_CHUNKS = {(0, 0), (0, 1), (0, 2), (0, 3), (1, 3)}
            ssum, ssq = {}, {}
            junkp = ctx.enter_context(tc.tile_pool(name="junk", bufs=2))
            x8src = x8T.rearrange("(a p) t -> p a t", p=128)
            for q in range(NQ):
                nc.sync.dma_start(
                    x8t[q][:].rearrange("p (a t) -> p a t", a=CT),
                    x8src[:, :, q * QTOK : (q + 1) * QTOK],
                )
                # HAM warm-up: dummy row-sum matmuls paced by the DMA
                # arrivals keep the PE's activity monitor at K=8/8 through
                # stage A, so stage B doesn't start at the 1.2 GHz cold clock.
                for k in range(8):
                    wps = psQuad.tile([1, 512], F32, tag="psQ", name=f"warm{q}{k}")
                    nc.tensor.matmul(
                        wps[:],
                        ones8[:],
                        x8t[q][:, k * 512 : (k + 1) * 512],
                        start=True,
                        stop=True,
                    )
                for t in range(CT):
                    sl = x8t[q][:, t * QTOK : (t + 1) * QTOK]
                    if (q, t) in SC_CHUNKS:
                        s1 = gnp.tile([128, 1], F32, tag=f"ss{q}{t}", name=f"ss{q}{t}")
                        s2 = gnp.tile([128, 1], F32, tag=f"sq{q}{t}", name=f"sq{q}{t}")
                        j1 = junkp.tile([128, QTOK], F32, tag="junk")
                        nc.scalar.activation(
                            j1[:], sl, AF.Identity, bias=zbias[:], accum_out=s1[:]
                        )
                        j2 = junkp.tile([128, QTOK], F32, tag="junk")
                        nc.scalar.activation(
                            j2[:], sl, AF.Square, bias=zbias[:], accum_out=s2[:]
                        )
                        ssum[q, t] = s1
                        ssq[q, t] = s2
                    else:
                        nc.vector.bn_stats(stats_t[t][:, 2 * q, :], sl[:, 0:512])
                        nc.vector.bn_stats(
                            stats_t[t][:, 2 * q + 1, :], sl[:, 512:1024]
                        )

            # weights land during the stats compute: one 3D DMA per tensor.
            # Issued BEFORE the residual xo (needed only at the epilogues) so
            # the rank1/scale/w8 chain isn't stuck behind a 4MB transfer.
            for n in ("wk", "wq", "wv", "wp"):
                wall = wpp.tile([128, CT * C], BF16, tag=f"w{n}")
                nc.sync.dma_start(
                    wall[:].rearrange("p (a c) -> p a c", a=CT),
                    w_ext[n].rearrange("(a p) c -> p a c", p=128),
                )
                w_r[n] = [wall[:, ci * C : (ci + 1) * C] for ci in range(CT)]
            wp_r = w_r["wp"]

            # residual x: one 3D DMA  [p, co, tok] <- xoT[co*128+p, tok]
            nc.sync.dma_start(
                xo_all[:].rearrange("p (a t) -> p a t", a=CT),
                xoT.rearrange("(a p) t -> p a t", p=128),
            )

            packed = gnp.tile([128, 2 * CT], F32, tag="packed")
            for t in range(CT):
                # merge DVE bn_stats (N_d tokens) with scalar accum sums
                nsc = sum(1 for q in range(NQ) if (q, t) in SC_CHUNKS)
                n_d = NTOK - nsc * QTOK
                # aggregate only the DVE-written slots (bn_aggr's variance
                # merge breaks on zero-count slots); scalar chunks are a
                # prefix of the quarters, so valid slots are contiguous.
                mv = gnp.tile([128, 2], F32, tag="mv")
                nc.vector.bn_aggr(mv[:], stats_t[t][:, 2 * nsc : 2 * NQ, :])
                tmp = gnp.tile([128, 1], F32, tag="tmp")
                nc.vector.tensor_mul(tmp[:], mv[:, 0:1], mv[:, 0:1])
                e2d = gnp.tile([128, 1], F32, tag="e2d")
                nc.vector.tensor_add(e2d[:], mv[:, 1:2], tmp[:])
                qs = [q for q in range(NQ) if (q, t) in SC_CHUNKS]
                s1, s2 = ssum[qs[0], t], ssq[qs[0], t]
                for q in qs[1:]:
                    s1b = gnp.tile([128, 1], F32, tag="s1b")
                    nc.vector.tensor_add(s1b[:], s1[:], ssum[q, t][:])
                    s2b = gnp.tile([128, 1], F32, tag="s2b")
                    nc.vector.tensor_add(s2b[:], s2[:], ssq[q, t][:])
                    s1, s2 = s1b, s2b
                ma = gnp.tile([128, 1], F32, tag="ma")
                nc.vector.tensor_scalar_mul(ma[:], mv[:, 0:1], float(n_d) / NTOK)
                mb = gnp.tile([128, 1], F32, tag="mb")
                nc.vector.tensor_scalar_mul(mb[:], s1[:], 1.0 / NTOK)
                nc.vector.tensor_add(packed[:, 2 * t : 2 * t + 1], ma[:], mb[:])
                ea = gnp.tile([128, 1], F32, tag="ea")
                nc.vector.tensor_scalar_mul(ea[:], e2d[:], float(n_d) / NTOK)
                eb = gnp.tile([128, 1], F32, tag="eb")
                nc.vector.tensor_scalar_mul(eb[:], s2[:], 1.0 / NTOK)
                nc.vector.tensor_add(
                    packed[:, 2 * t + 1 : 2 * t + 2], ea[:], eb[:]
                )

            g_ps = psQuad.tile([8, 2 * CT], F32, tag="psQ", name="g_ps")
            nc.tensor.matmul(g_ps[:], gsel_sb[:], packed[:], start=True, stop=True)
            stat2 = gnp.tile([8, 2 * CT], F32, tag="stat2")
            nc.vector.tensor_scalar_mul(stat2[:], g_ps[:], 1.0 / CG)
            s2v = stat2.rearrange("g (t two) -> g t two", two=2)
            mu_v = s2v[:, :, 0]
            e2_v = s2v[:, :, 1]
            musq = gnp.tile([8, CT], F32, tag="musq")
            nc.vector.tensor_mul(musq[:], mu_v, mu_v)
            var = gnp.tile([8, CT], F32, tag="var")
            nc.vector.tensor_sub(var[:], e2_v, musq[:])
            sqv = gnp.tile([8, CT], F32, tag="sqv")
            nc.scalar.activation(sqv[:], var[:], AF.Sqrt, bias=eps_row[:], scale=1.0)
            rstd = gnp.tile([8, CT], F32, tag="rstd")
            nc.vector.reciprocal(rstd[:], sqv[:])

            scale_t, shift_t, shift_bf = [], [], []
            for t in range(CT):
                cat2 = gnp.tile([8, 2], F32, tag="cat2")
                nc.vector.tensor_copy(cat2[:, 0:1], mu_v[:, t : t + 1])
                nc.vector.tensor_copy(cat2[:, 1:2], rstd[:, t : t + 1])
                bc_ps = psQuad.tile([128, 2], F32, tag="psQ", name=f"bc{t}")
                nc.tensor.matmul(bc_ps[:], gselT_sb[:], cat2[:], start=True, stop=True)
                sc = gnp.tile([128, 1], F32, tag=f"scale{t}")
                nc.vector.tensor_mul(sc[:], bc_ps[:, 1:2], gamma_t[t])
                tmp2 = gnp.tile([128, 1], F32, tag="tmp2")
                nc.vector.tensor_mul(tmp2[:], bc_ps[:, 0:1], sc[:])
                sh = gnp.tile([128, 1], F32, tag=f"shift{t}")
                nc.vector.tensor_sub(sh[:], beta_t[t], tmp2[:])
                shb = gnp.tile([128, 1], BF16, tag=f"shiftb{t}")
                nc.vector.tensor_copy(shb[:], sh[:])
                scale_t.append(sc)
                shift_t.append(sh)
                shift_bf.append(shb)

            # ---- fold groupnorm into the projections ------------------------
            # xn = s*x + t  =>  xn @ w = x @ (diag(s) w) + (t @ w).

            def rank1_bias(wname, b_tiles, shvec, tag):
                """per-co bias tiles: b[co] + sum_ci shvec[ci] @ w[ci, co]"""
                out = []
                for co in range(CT):
                    ps = psQuad.tile([128, 1], F32, tag="psQ", name=f"r1{tag}{co}")
                    for ci in range(CT):
                        nc.tensor.matmul(
                            ps[:],
                            w_r[wname][ci][:, co * 128 : (co + 1) * 128],
                            shvec[ci][:],
                            start=(ci == 0),
                            stop=(ci == CT - 1),
                        )
                    bt = smalls.tile([128, 1], F32, tag=f"bfold{tag}{co}")
                    nc.vector.tensor_add(bt[:], b_tiles[co], ps[:])
                    out.append(bt)
                return out

            biask_t = rank1_bias("wk", bk_t, shift_bf, "k")
            biasq_t = rank1_bias("wq", bq_t, shift_bf, "q")
            # V bias rides through the softmax (weights sum to 1):
            # bp'' = bp + (bv + t @ wv) @ wp
            bvp_t = rank1_bias("wv", bv_t, shift_bf, "v")
            bvp_bf = []
            for ci in range(CT):
                bb = gnp.tile([128, 1], BF16, tag=f"bvpb{ci}")
                nc.vector.tensor_copy(bb[:], bvp_t[ci][:])
                bvp_bf.append(bb)
            bpp_t = rank1_bias("wp", bp_t, bvp_bf, "p")
            # fold the (bp + bv'@wp) bias into the residual once, so the
            # per-block epilogue needs only y = y1 + xr' (plain add, no STT).
            # On the DVE: the scalar FIFO feeds stage B's K/Q activations next.
            for co in range(CT):
                sl = xo_all[:, co * NOWN : (co + 1) * NOWN]
                nc.vector.tensor_scalar_add(sl, sl, bpp_t[co][:])

            # in-place row-scale of wq/wk/wv (after all rank-1 reads), then
            # fp8 DoubleRow conversion: w8[n][ci2] flat [p, i*512 + co];
            # input channel = ci2*256 + i*128 + p. wk goes first so stage B's
            # K matmuls can start while wq/wv are still being converted.
            w8 = {}

            def scale_w(n):
                for ci in range(CT):
                    nc.vector.tensor_scalar_mul(
                        w_r[n][ci][:], w_r[n][ci][:], scale_t[ci][:]
                    )

            def conv_w8(n):
                w8[n] = []
                for c in range(2):
                    t8 = wpp.tile([128, 1024], FP8, tag=f"w8{n}{c}")
                    for i in range(2):
                        nc.scalar.activation(
                            t8[:, i * 512 : (i + 1) * 512],
                            w_r[n][2 * c + i][:],
                            AF.Identity,
                            bias=zbias[:],
                            scale=1.0,
                        )
                    w8[n].append(t8)

            scale_w("wk")
            conv_w8("wk")
            scale_w("wq")
            conv_w8("wq")
            scale_w("wv")

        # ---- stage B: QKV projections (K/Q fp8 DoubleRow, V bf16) ------------
        with nc.named_scope("qkv"):
            w8v = {
                n: [
                    w8[n][c][:].rearrange("p (two co) -> p two co", two=2)
                    for c in range(2)
                ]
                for n in ("wk", "wq")
            }
            x8v = {
                (q, c): x8t[q][:, c * 2 * QTOK : (c + 1) * 2 * QTOK].rearrange(
                    "p (two t) -> p two t", two=2
                )
                for q in range(NQ)
                for c in range(2)
            }
            for q in range(NQ):
                # K^T: per co, both 512-chunks into one [128,1024] psum
                for co in range(CT):
                    psB = psBig.tile([128, 1024], F32, tag="psB")
                    for nch in range(2):
                        for c in range(2):
                            nc.tensor.matmul(
                                psB[:, nch * 512 : (nch + 1) * 512],
                                w8v["wk"][c][:, :, co * 128 : (co + 1) * 128],
                                x8v[q, c][:, :, nch * 512 : (nch + 1) * 512],
                                start=(c == 0),
                                stop=(c == 1),
                                perf_mode=DR,
                            )
                    ci2, half = co // 2, co % 2
                    dst = KT8[ci2][
                        :, half * NTOK + q * QTOK : half * NTOK + (q + 1) * QTOK
                    ]
                    nc.scalar.activation(
                        dst, psB[:], AF.Identity, bias=biask_t[co][:], scale=1.0
                    )
                # Q^T: own-half quarters only
                if q < 2:
                    for co in range(CT):
                        psB = psBig.tile([128, 1024], F32, tag="psB")
                        for nch in range(2):
                            for c in range(2):
                                nc.tensor.matmul(
                                    psB[:, nch * 512 : (nch + 1) * 512],
                                    w8v["wq"][c][:, :, co * 128 : (co + 1) * 128],
                                    x8v[q, c][:, :, nch * 512 : (nch + 1) * 512],
                                    start=(c == 0),
                                    stop=(c == 1),
                                    perf_mode=DR,
                                )
                        ci2, half = co // 2, co % 2
                        dst = Q8[ci2][
                            :, half * NOWN + q * QTOK : half * NOWN + (q + 1) * QTOK
                        ]
                        nc.scalar.activation(
                            dst, psB[:], AF.Identity, bias=biasq_t[co][:], scale=1.0
                        )
                # V: two 128-token tiles into one [128,1024] psum -> V8[pair]
                # lhsT is fp8 x with bf16 wv rhs (mixed non-fp32 dtypes)
                for jt2 in range(4):
                    psB = psBig.tile([128, 1024], F32, tag="psB")
                    for half2 in range(2):
                        jt = jt2 * 2 + half2
                        for ci in range(CT):
                            nc.tensor.matmul(
                                psB[:, half2 * 512 : (half2 + 1) * 512],
                                x8t[q][
                                    :,
                                    ci * QTOK + jt * 128 : ci * QTOK + (jt + 1) * 128,
                                ],
                                w_r["wv"][ci][:],
                                start=(ci == 0),
                                stop=(ci == CT - 1),
                            )
                    pair = q * 4 + jt2
                    # on the DVE: keeps the scalar FIFO free so the first
                    # attention exps aren't starved behind stage-B activations
                    nc.vector.tensor_copy(V8[pair][:], psB[:])

        # ---- stage C: attention + projection ---------------------------------
        with (
            tc.tile_pool(name="pt", bufs=8) as ptp,
            tc.tile_pool(name="osb", bufs=4) as osbp,
            tc.tile_pool(name="ysb", bufs=3) as ysbp,
            tc.tile_pool(name="yraw", bufs=4) as yrawp,
            tc.tile_pool(name="racc", bufs=2) as raccp,
            tc.tile_pool(name="rsb", bufs=2) as rsbp,
            nc.named_scope("attn"),
        ):
            kt3 = [
                KT8[c][:].rearrange("p (two j) -> p two j", two=2) for c in range(2)
            ]
            v3 = [
                V8[j][:].rearrange("p (two c) -> p two c", two=2)
                for j in range(NPAIR)
            ]

            def emit_o(state, pair, pt_t):
                # psO tiles are allocated lazily at the first emit_o so the
                # previous block's epilogue PSUM allocations (emitted at
                # pair==1) precede them in pool order — otherwise the pool's
                # FIFO buffer reuse creates an allocation-order deadlock.
                if state["psO_t"] is None:
                    ib = state["ib"]
                    state["psO_t"] = [
                        psQuad.tile([128, 512], F32, tag="psQ", name=f"psO_{ib}_{i}")
                        for i in range(CT)
                    ]
                psO_t = state["psO_t"]
                pt3 = pt_t[:].rearrange("p (two n) -> p two n", two=2)
                for ct in range(CT):
                    nc.tensor.matmul(
                        psO_t[ct][:],
                        v3[pair][:, :, ct * 128 : (ct + 1) * 128],
                        pt3,
                        start=(pair == 0),
                        stop=(pair == NPAIR - 1),
                        perf_mode=DR,
                    )

            def emit_epilogue_a(state):
                """r chain + O drain + projection for a finished block.

                Engine placement is deliberate: osb/yraw go on the DVE (they
                are data-ready when emitted; on the scalar FIFO they would
                delay the next block's exp), rinv uses the fast approx so it
                finishes before the PE reaches the rb broadcast matmul."""
                ib = state["ib"]
                racc, psO_t = state["racc"], state["psO_t"]

                psr = psQuad.tile([1, 512], F32, tag="psQ", name=f"psr{ib}")
                nc.tensor.matmul(psr[:], ones_r[:], racc[:], start=True, stop=False)
                nc.tensor.matmul(
                    psr[:], ones_r[:], state["racc2"][:], start=False, stop=True
                )

                osb = []
                for ct in range(CT):
                    o_t = osbp.tile([128, 512], BF16, tag="osb")
                    nc.scalar.activation(
                        o_t[:], psO_t[ct][:], AF.Identity, bias=zbias[:], scale=1.0
                    )
                    osb.append(o_t)

                rinv = rsbp.tile([1, 512], F32, tag="rinv")
                rscratch = rsbp.tile([1, 512], F32, tag="rscr")
                nc.vector.reciprocal_approx_accurate(
                    rinv[:], psr[:], rscratch[:]
                )

                yraw = []
                for co in range(CT):
                    psY = psQuad.tile([128, 512], F32, tag="psQ", name=f"psY{ib}{co}")
                    for ci in range(CT):
                        nc.tensor.matmul(
                            psY[:],
                            wp_r[ci][:, co * 128 : (co + 1) * 128],
                            osb[ci][:],
                            start=(ci == 0),
                            stop=(ci == CT - 1),
                        )
                    yr = yrawp.tile([128, 512], F32, tag="yraw")
                    nc.scalar.activation(
                        yr[:], psY[:], AF.Identity, bias=zbias[:], scale=1.0
                    )
                    yraw.append(yr)
                state["yraw"] = yraw

                rb_ps = psQuad.tile([128, 512], F32, tag="psQ", name=f"rb{ib}")
                nc.tensor.matmul(rb_ps[:], ones1_f[:], rinv[:], start=True, stop=True)
                rb_sb = rsbp.tile([128, 512], F32, tag="rb_sb")
                nc.vector.tensor_copy(rb_sb[:], rb_ps[:])
                state["rb_sb"] = rb_sb

            def emit_epilogue_b(state):
                """normalize + bias + residual + output DMA (rb surely ready)."""
                ib = state["ib"]
                i0 = ib * 512
                rb_sb, yraw = state["rb_sb"], state["yraw"]
                for co in range(CT):
                    xr = xo_all[:, co * NOWN + i0 : co * NOWN + i0 + 512]
                    y1_t = ysbp.tile([128, 512], F32, tag="y1sb")
                    nc.vector.tensor_mul(y1_t[:], yraw[co][:], rb_sb[:])
                    y_t = ysbp.tile([128, 512], F32, tag="ysb")
                    nc.vector.tensor_add(y_t[:], y1_t[:], xr)
                    nc.sync.dma_start(
                        yT_ext[co * 128 : (co + 1) * 128, i0 : i0 + 512], y_t[:]
                    )

            # 2-deep software pipeline ACROSS block boundaries: the last two
            # O groups of block b interleave with block b+1's first S groups,
            # so the PE never runs an S-only (exp-gated) stretch.
            done_state = None
            pending = []  # [(state, pair, pt_t)]
            for ib in range(IB):
                qrhs = [
                    Q8[c][:].rearrange("p (two n) -> p two n", two=2)[
                        :, :, ib * 512 : (ib + 1) * 512
                    ]
                    for c in range(2)
                ]
                state = {
                    "ib": ib,
                    "psO_t": None,
                    "racc": raccp.tile([128, 512], F32R, tag="racc", name=f"racc{ib}"),
                    "racc2": raccp.tile(
                        [128, 512], F32R, tag="racc2", name=f"racc2_{ib}"
                    ),
                }
                racc = state["racc"]

                for pair in range(NPAIR):
                    psS2 = psBig.tile([128, 1024], F32, tag="psB")
                    for half in range(2):
                        j = pair * 2 + half
                        for ci2 in range(2):
                            nc.tensor.matmul(
                                psS2[:, half * 512 : (half + 1) * 512],
                                kt3[ci2][:, :, j * 128 : (j + 1) * 128],
                                qrhs[ci2],
                                start=(ci2 == 0),
                                stop=(ci2 == 1),
                                perf_mode=DR,
                            )
                    # epilogue BEFORE the pop: its PSUM allocations (psr/psY/rb)
                    # must precede the next block's psO allocations in pool
                    # order, or the pool's FIFO buffer reuse stalls the PE a
                    # whole block waiting on them.
                    if pair == 2 and done_state is not None:
                        # previous block's epilogue (its last O landed at pair 1)
                        emit_epilogue_a(done_state)
                    elif pair == 7 and done_state is not None:
                        emit_epilogue_b(done_state)
                        done_state = None
                    if len(pending) >= 2:
                        emit_o(*pending.pop(0))
                    pt_t = ptp.tile([128, 1024], FP8, tag="pt")
                    nc.scalar.activation(
                        pt_t[:], psS2[:], AF.Exp, bias=ebias[:], scale=SCALE
                    )
                    # two running sums, one per engine, merged in the epilogue:
                    # DVE accumulates half 0, GpSimd half 1 (fp8 reads)
                    racc2 = state["racc2"]
                    if pair == 0:
                        nc.vector.tensor_copy(racc[:], pt_t[:, 0:512])
                        nc.gpsimd.tensor_copy(racc2[:], pt_t[:, 512:1024])
                    else:
                        nc.vector.tensor_add(racc[:], racc[:], pt_t[:, 0:512])
                        nc.gpsimd.tensor_add(racc2[:], racc2[:], pt_t[:, 512:1024])
                    pending.append((state, pair, pt_t))
                done_state = state
            for item in pending:
                emit_o(*item)
            emit_epilogue_a(done_state)
            emit_epilogue_b(done_state)

    nc.compile()
    _CACHE["nc"] = nc
    return nc


def make_in_maps(x, gamma, beta, wq, bq, wk, bk, wv, bv, wp, bp):
    import ml_dtypes

    bf16 = ml_dtypes.bfloat16
    x = np.asarray(x, dtype=np.float32)
    gsel = np.zeros((128, 8), np.float32)
    for p in range(128):
        gsel[p, p // CG % 8] = 1.0
    gselT = np.ascontiguousarray(gsel.T)

    shared = {
        "wq": np.asarray(wq, np.float32).astype(bf16),
        "wk": np.asarray(wk, np.float32).astype(bf16),
        "wv": np.asarray(wv, np.float32).astype(bf16),
        "wp": np.asarray(wp, np.float32).astype(bf16),
        "bq": np.asarray(bq, np.float32),
        "bk": np.asarray(bk, np.float32),
        "bv": np.asarray(bv, np.float32),
        "bp": np.asarray(bp, np.float32),
        "gamma": np.asarray(gamma, np.float32),
        "beta": np.asarray(beta, np.float32),
        "gsel": gsel,
        "gselT": gselT,
    }

    in_maps = []
    for core in range(8):
        b, h = core // 2, core % 2
        xT_b = np.ascontiguousarray(x[b].reshape(NTOK, C).T)  # [C, NTOK]
        if h == 1:
            xT_b = np.ascontiguousarray(
                np.concatenate([xT_b[:, NOWN:], xT_b[:, :NOWN]], axis=1)
            )
        in_maps.append(
            {
                # |x| < 240, so OCP e4m3fn bytes == TRN fp8e4 bytes
                "x8T": xT_b.astype(ml_dtypes.float8_e4m3fn),
                "xoT": np.ascontiguousarray(xT_b[:, :NOWN]),
                **shared,
            }
        )
    return in_maps


def kernel(x, gamma, beta, wq, bq, wk, bk, wv, bv, wp, bp):
    nc = _build_nc()
    in_maps = make_in_maps(x, gamma, beta, wq, bq, wk, bk, wv, bv, wp, bp)
    _CACHE["in_maps"] = in_maps

    res = run_bass_kernel_spmd(nc, in_maps, core_ids=list(range(8)))

    y = np.empty((B, NTOK, C), np.float32)
    for core in range(8):
        b, h = core // 2, core % 2
        yT = res.results[core]["yT"]  # [C, NOWN]
        y[b, h * NOWN : (h + 1) * NOWN, :] = yT.T
    return y.reshape(B, HH, WW, C)
